# revision 1
# baseline (speedup 1.0000x reference)
"""Trainium2 Bass kernel for nn_CrossAttention (B=2, S=2048, E=1024, H=16, ctx=768).

Sharding: 4-way tensor-parallel over heads x 2-way data-parallel over batch.
Core c handles batch c//4 and heads 4*(c%4) .. 4*(c%4)+3.

Per-core dataflow (all matmuls fp16 operands, fp32 PSUM accumulate):
  qT/kT = W-stationary projections producing [dh, S] layouts directly
  v     = ctxT-tile-stationary projection producing natural [S, dh]
  scT   = kT-tile x qT (K=64); the two heads of a pair are emitted
          back-to-back on PE row groups 0/64 so they can run concurrently
  exp   = ScalarE, fused 1/sqrt(dh) scale, PSUM -> SBUF fp16
  av/Z  = v_h (cols 0:64) and ones (cols 64:128) col-packed into one
          PSUM bank: rows 0:64 = unnormalized out.T, rows 64:128 =
          softmax denominator replicated.  Normalized via a base-0 SBUF
          staged reciprocal_approx_fast (custom DVE ops require SBUF
          inputs at partition base 0) + mixed-base multiply.
  out   = avT-stationary x Wo, partial [S, E] per core

Host side: pre-transpose x/context, slice weights per head group, fp16 cast;
sum the 4 per-batch partials + bo on host.
"""
import numpy as np

import concourse.bass as bass
import concourse.mybir as mybir
import concourse.tile as tile
from concourse import bacc, bass_utils

F16 = mybir.dt.float16
F32 = mybir.dt.float32
AF = mybir.ActivationFunctionType
OP = mybir.AluOpType

B, S, E, C, H, DH = 2, 2048, 1024, 768, 16, 64
N_CORES = 8
GROUPS = 4            # head groups (tensor parallel)
HPG = H // GROUPS     # heads per group = 4
DSL = HPG * DH        # feature slice per core = 256
KT_E = E // 128       # 8 k-tiles for x projections
KT_C = C // 128       # 6 k-tiles for context projections
SCK = S // 512        # 4 s-chunks
TT = S // 128         # 16 t-tiles

_NC_CACHE = {}


def _build_nc():
    nc = bacc.Bacc("TRN2", target_bir_lowering=False, debug=False,
                   num_devices=N_CORES)

    xT = nc.dram_tensor("xT", [E, S], F16, kind="ExternalInput").ap()
    ctxT = nc.dram_tensor("ctxT", [C, S], F16, kind="ExternalInput").ap()
    wq = nc.dram_tensor("wq", [E, DSL], F16, kind="ExternalInput").ap()
    wk = nc.dram_tensor("wk", [C, DSL], F16, kind="ExternalInput").ap()
    wv = nc.dram_tensor("wv", [C, DSL], F16, kind="ExternalInput").ap()
    wo = nc.dram_tensor("wo", [DSL, E], F16, kind="ExternalInput").ap()
    bq = nc.dram_tensor("bq", [128, 2], F32, kind="ExternalInput").ap()
    bk = nc.dram_tensor("bk", [128, 2], F32, kind="ExternalInput").ap()
    bv = nc.dram_tensor("bv", [1, DSL], F16, kind="ExternalInput").ap()
    out = nc.dram_tensor("out", [S, E], F32, kind="ExternalOutput").ap()

    xT_r = xT.rearrange("(o p) s -> p o s", p=128)
    ctxT_r = ctxT.rearrange("(o p) s -> p o s", p=128)

    with tile.TileContext(nc) as tc:
        with (
            tc.tile_pool(name="const", bufs=1) as cpool,
            tc.tile_pool(name="qkv", bufs=1) as qpool,
            tc.tile_pool(name="ex", bufs=6) as expool,
            tc.tile_pool(name="os", bufs=3) as ospool,
        ):
            wq_sb = cpool.tile([128, KT_E, DSL], F16)
            wk_sb = cpool.tile([128, KT_C, DSL], F16)
            wv_sb = cpool.tile([128, KT_C, DSL], F16)
            wo_sb = cpool.tile([128, 2, E], F16)
            bq_sb = cpool.tile([128, 2], F32)
            bk_sb = cpool.tile([128, 2], F32)
            bv_sb = cpool.tile([1, DSL], F16)
            ones_sb = cpool.tile([128, DH], F16)
            onesr_sb = cpool.tile([1, 128], F16)
            warm_sb = cpool.tile([1, 8], F32)
            ctxT_sb = cpool.tile([128, KT_C, S], F16)
            xT_sb = cpool.tile([128, KT_E, S], F16)

            nc.sync.dma_start(wk_sb[:], wk.rearrange("(o p) m -> p o m", p=128))
            nc.sync.dma_start(bk_sb[:], bk[:])
            for k in range(KT_C):
                nc.sync.dma_start(ctxT_sb[:, k, :], ctxT_r[:, k, :])
            nc.sync.dma_start(wv_sb[:], wv.rearrange("(o p) m -> p o m", p=128))
            nc.sync.dma_start(bv_sb[:], bv[:])
            nc.sync.dma_start(wq_sb[:], wq.rearrange("(o p) m -> p o m", p=128))
            nc.sync.dma_start(bq_sb[:], bq[:])
            nc.vector.memset(ones_sb[:], 1.0)
            nc.vector.memset(onesr_sb[:], 1.0)
            nc.vector.memset(warm_sb[:], 0.0)
            # pull the exp table load off the critical path
            nc.scalar.activation(warm_sb[:], warm_sb[:], AF.Exp)
            for k in range(KT_E):
                nc.sync.dma_start(xT_sb[:, k, :], xT_r[:, k, :])
            nc.sync.dma_start(wo_sb[:], wo.rearrange("(l p) n -> p l n", p=128))

            qT_sb = qpool.tile([128, 2, S], F16)
            kT_sb = qpool.tile([128, 2, S], F16)
            # per (t, head): 128 cols = [v_h (64) | ones (64)] so one matmul
            # yields av rows 0:64 and the replicated softmax denominator
            # rows 64:128 in a single PSUM bank
            v2_sb = qpool.tile([128, TT, HPG, 128], F16)
            avT_sb = qpool.tile([128, 2, S], F16)
            nc.vector.memset(v2_sb[:], 1.0)

            # ---- single pool set (no phase boundary); projections are
            # emitted layer-0-first so the first attention head-pair's
            # inputs are ready early and exp overlaps the remaining proj ----
            with (
                tc.tile_pool(name="psc", bufs=3, space="PSUM") as psc,
                tc.tile_pool(name="pavz", bufs=2, space="PSUM") as pavz,
            ):
                def proj_qk(dst, w_sb, b_sb, src, nk, l):
                    for g in range(2):
                        pss = [psc.tile([128, 512], F32, tag="sc",
                                        name=f"pj{dst.tensor.name}_{l}_{g}_{i}")
                               for i in range(2)]
                        for k in range(nk):
                            for i in range(2):
                                sc = g * 2 + i
                                nc.tensor.matmul(
                                    pss[i][:],
                                    w_sb[:, k, l * 128:(l + 1) * 128],
                                    src[:, k, sc * 512:(sc + 1) * 512],
                                    start=(k == 0), stop=(k == nk - 1),
                                )
                        for i in range(2):
                            sc = g * 2 + i
                            nc.vector.tensor_tensor(
                                dst[:, l, sc * 512:(sc + 1) * 512],
                                pss[i][:],
                                b_sb[:, l:l + 1].to_broadcast([128, 512]),
                                OP.add,
                            )

                proj_qk(kT_sb, wk_sb, bk_sb, ctxT_sb, KT_C, 0)
                proj_qk(qT_sb, wq_sb, bq_sb, xT_sb, KT_E, 0)

                for t in range(TT):
                    ps = pavz.tile([128, DSL], F32, tag="avz",
                                   name=f"vps{t}")
                    for k in range(KT_C):
                        nc.tensor.matmul(
                            ps[:],
                            ctxT_sb[:, k, t * 128:(t + 1) * 128],
                            wv_sb[:, k, :],
                            start=(k == 0), stop=False,
                        )
                    nc.tensor.matmul(
                        ps[:], onesr_sb[:, :], bv_sb[:, :],
                        start=False, stop=True,
                    )
                    nc.vector.tensor_copy(
                        v2_sb[:, t, :, 0:DH],
                        ps[:].rearrange("p (g d) -> p g d", d=DH),
                    )

                proj_qk(kT_sb, wk_sb, bk_sb, ctxT_sb, KT_C, 1)
                proj_qk(qT_sb, wq_sb, bq_sb, xT_sb, KT_E, 1)

                # ---- attention + output projection ----
                def outproj(sc):
                    for st in range(4):
                        row = (sc * 4 + st) * 128
                        psos = [psc.tile([128, 512], F32, tag="sc",
                                         name=f"po{sc}_{st}_{n}")
                                for n in range(2)]
                        for l in range(2):
                            for n in range(2):
                                nc.tensor.matmul(
                                    psos[n][:],
                                    avT_sb[:, l, row:row + 128],
                                    wo_sb[:, l, n * 512:(n + 1) * 512],
                                    start=(l == 0), stop=(l == 1),
                                )
                        for n in range(2):
                            os_sb = ospool.tile([128, 512], F32, tag="os")
                            nc.vector.tensor_copy(os_sb[:], psos[n][:])
                            nc.sync.dma_start(
                                out[row:row + 128, n * 512:(n + 1) * 512],
                                os_sb[:],
                            )

                for sc in range(SCK):
                    ssl = slice(sc * 512, (sc + 1) * 512)
                    for p in range(2):
                        avz = {(p, h): pavz.tile([128, 512], F32, tag="avz",
                                                 name=f"avz{sc}_{p}_{h}")
                               for h in range(2)}
                        for t in range(TT):
                            # both heads in one 2-bank tile: h0 cols 0:512,
                            # h1 cols 512:1024 — the pair is emitted
                            # back-to-back on PE row groups 0/64
                            scp = psc.tile([128, 1024], F32, tag="sc",
                                           name=f"sc{sc}_{t}_{p}")
                            for h in range(2):
                                hb = h * DH
                                nc.tensor.matmul(
                                    scp[:, h * 512:(h + 1) * 512],
                                    kT_sb[hb:hb + DH, p,
                                          t * 128:(t + 1) * 128],
                                    qT_sb[hb:hb + DH, p, ssl],
                                    start=True, stop=True,
                                )
                            ex = expool.tile([128, 1024], F16, tag="ex",
                                             name=f"ex{sc}_{t}_{p}")
                            nc.scalar.activation(ex[:], scp[:], AF.Exp,
                                                 scale=0.125)
                            # single [v_h | ones] stationary: av rows 0:64,
                            # replicated Z rows 64:128
                            for h in range(2):
                                nc.tensor.matmul(
                                    avz[p, h][:, :],
                                    v2_sb[:, t, p * 2 + h, :],
                                    ex[:, h * 512:(h + 1) * 512],
                                    start=(t == 0), stop=(t == TT - 1),
                                )
                        for h in range(2):
                            hb = h * DH
                            # custom DVE op: SBUF-only, partition base 0
                            rz = ospool.tile([128, 1024], F32, tag="rz",
                                             name=f"rz{sc}_{p}_{h}")
                            nc.vector.tensor_copy(
                                rz[0:DH, 0:512], avz[p, h][DH:128, :])
                            nc.vector.reciprocal_approx_fast(
                                rz[0:DH, 512:1024], rz[0:DH, 0:512])
                            nc.vector.tensor_tensor(
                                avT_sb[hb:hb + DH, p, ssl],
                                avz[p, h][0:DH, :],
                                rz[0:DH, 512:1024],
                                OP.mult,
                            )
                    # defer the PREVIOUS chunk's output projection into
                    # this chunk's ACT-bound attention stretch
                    if sc > 0:
                        outproj(sc - 1)
                outproj(SCK - 1)

    nc.compile()
    return nc


def get_nc():
    if "nc" not in _NC_CACHE:
        _NC_CACHE["nc"] = _build_nc()
    return _NC_CACHE["nc"]


def make_in_maps(x, context, Wq, bq, Wk, bk, Wv, bv, Wo, bo):
    x = np.asarray(x, dtype=np.float32)
    context = np.asarray(context, dtype=np.float32)
    Wq = np.asarray(Wq, dtype=np.float32)
    Wk = np.asarray(Wk, dtype=np.float32)
    Wv = np.asarray(Wv, dtype=np.float32)
    Wo = np.asarray(Wo, dtype=np.float32)
    bq = np.asarray(bq, dtype=np.float32)
    bk = np.asarray(bk, dtype=np.float32)
    bv = np.asarray(bv, dtype=np.float32)

    xT = [np.ascontiguousarray(x[b].T).astype(np.float16) for b in range(B)]
    ctxT = [np.ascontiguousarray(context[b].T).astype(np.float16)
            for b in range(B)]
    in_maps = []
    for c in range(N_CORES):
        b, g = c // GROUPS, c % GROUPS
        sl = slice(g * DSL, (g + 1) * DSL)
        in_maps.append({
            "xT": xT[b],
            "ctxT": ctxT[b],
            "wq": Wq[:, sl].astype(np.float16),
            "wk": Wk[:, sl].astype(np.float16),
            "wv": Wv[:, sl].astype(np.float16),
            "wo": Wo[sl, :].astype(np.float16),
            "bq": np.ascontiguousarray(bq[sl].reshape(2, 128).T),
            "bk": np.ascontiguousarray(bk[sl].reshape(2, 128).T),
            "bv": bv[sl].reshape(1, DSL).astype(np.float16),
        })
    return in_maps


def run_sharded(inputs, trace=False):
    nc = get_nc()
    in_maps = make_in_maps(**inputs)
    res = bass_utils.run_bass_kernel_spmd(
        nc, in_maps, core_ids=list(range(N_CORES)), trace=trace,
    )
    bo = np.asarray(inputs["bo"], dtype=np.float32)
    full = np.empty((B, S, E), dtype=np.float32)
    for b in range(B):
        acc = res.results[b * GROUPS]["out"].astype(np.float32)
        for g in range(1, GROUPS):
            acc = acc + res.results[b * GROUPS + g]["out"]
        full[b] = acc + bo[None, :]
    return full, res.exec_time_ns


def kernel(**inputs) -> np.ndarray:
    return run_sharded(inputs)[0]



# revision 2
# speedup vs baseline: 1.1292x; 1.1292x over previous
"""Trainium2 Bass kernel for nn_CrossAttention (B=2, S=2048, E=1024, H=16, ctx=768).

Sharding: 4-way tensor-parallel over heads x 2-way data-parallel over batch.
Core c handles batch c//4 and heads 4*(c%4) .. 4*(c%4)+3.

Per-core dataflow (fp16 operands, fp32 PSUM accumulate), software-pipelined
so ScalarE (exp, the throughput floor at ~1us per [128,1024] tile) starts
~13us in and never starves:

  segments = head-pair-major: (p=0, sc=0..3) then (p=1, sc=0..3).
  Per segment t-iteration: score pair (concurrent K=64 matmuls on PE row
  groups 0/64) -> exp (ScalarE) banked into a deep SBUF ex pool, while the
  PREVIOUS segment's AV matmuls consume its banked ex tiles, plus one
  budgeted "filler" PE op (remaining projections / deferred out-proj).
  p=0 needs only layer-0 projections, so kT/qT layer-1 and the V
  projection run as fillers inside the p=0 segments.

  av/Z: v_h (cols 0:64) and ones (cols 64:128) col-packed into one PSUM
  bank: rows 0:64 = unnormalized out.T, rows 64:128 = softmax denominator.
  Normalized via base-0 SBUF staged reciprocal_approx_fast + mixed-base
  multiply.  Out-proj partials DMA'd as fp16; host sums partials + bo.
"""
import numpy as np

import concourse.bass as bass
import concourse.mybir as mybir
import concourse.tile as tile
from concourse import bacc, bass_utils

F16 = mybir.dt.float16
F32 = mybir.dt.float32
AF = mybir.ActivationFunctionType
OP = mybir.AluOpType

B, S, E, C, H, DH = 2, 2048, 1024, 768, 16, 64
N_CORES = 8
GROUPS = 4            # head groups (tensor parallel)
HPG = H // GROUPS     # heads per group = 4
DSL = HPG * DH        # feature slice per core = 256
KT_E = E // 128       # 8 k-tiles for x projections
KT_C = C // 128       # 6 k-tiles for context projections
SCK = S // 512        # 4 s-chunks
TT = S // 128         # 16 t-tiles

_NC_CACHE = {}


def _build_nc():
    nc = bacc.Bacc("TRN2", target_bir_lowering=False, debug=False,
                   num_devices=N_CORES)

    xT = nc.dram_tensor("xT", [E, S], F16, kind="ExternalInput").ap()
    ctxT = nc.dram_tensor("ctxT", [C, S], F16, kind="ExternalInput").ap()
    wq = nc.dram_tensor("wq", [E, DSL], F16, kind="ExternalInput").ap()
    wk = nc.dram_tensor("wk", [C, DSL], F16, kind="ExternalInput").ap()
    wv = nc.dram_tensor("wv", [C, DSL], F16, kind="ExternalInput").ap()
    wo = nc.dram_tensor("wo", [DSL, E], F16, kind="ExternalInput").ap()
    bq = nc.dram_tensor("bq", [128, 2], F32, kind="ExternalInput").ap()
    bk = nc.dram_tensor("bk", [128, 2], F32, kind="ExternalInput").ap()
    bv = nc.dram_tensor("bv", [1, DSL], F16, kind="ExternalInput").ap()
    out = nc.dram_tensor("out", [S, E], F16, kind="ExternalOutput").ap()

    xT_r = xT.rearrange("(o p) s -> p o s", p=128)
    ctxT_r = ctxT.rearrange("(o p) s -> p o s", p=128)

    with tile.TileContext(nc) as tc:
        with (
            tc.tile_pool(name="const", bufs=1) as cpool,
            tc.tile_pool(name="qkv", bufs=1) as qpool,
            tc.tile_pool(name="ex", bufs=20) as expool,
            tc.tile_pool(name="os", bufs=3) as ospool,
        ):
            wq_sb = cpool.tile([128, KT_E, DSL], F16)
            wk_sb = cpool.tile([128, KT_C, DSL], F16)
            wv_sb = cpool.tile([128, KT_C, DSL], F16)
            wo_sb = cpool.tile([128, 2, E], F16)
            bq_sb = cpool.tile([128, 2], F32)
            bk_sb = cpool.tile([128, 2], F32)
            bv_sb = cpool.tile([1, DSL], F16)
            onesr_sb = cpool.tile([1, 128], F16)
            warm_sb = cpool.tile([1, 8], F32)
            ctxT_sb = cpool.tile([128, KT_C, S], F16)
            xT_sb = cpool.tile([128, KT_E, S], F16)

            # DMA in first-needed-first order: kT proj deps, V deps, qT
            # deps, then xT in s-chunk column blocks so qT l0 sc0 can
            # start early; wo last.
            nc.sync.dma_start(wk_sb[:], wk.rearrange("(o p) m -> p o m", p=128))
            nc.sync.dma_start(bk_sb[:], bk[:])
            for k in range(KT_C):
                nc.sync.dma_start(ctxT_sb[:, k, :], ctxT_r[:, k, :])
            nc.sync.dma_start(wv_sb[:], wv.rearrange("(o p) m -> p o m", p=128))
            nc.sync.dma_start(bv_sb[:], bv[:])
            nc.sync.dma_start(wq_sb[:], wq.rearrange("(o p) m -> p o m", p=128))
            nc.sync.dma_start(bq_sb[:], bq[:])
            nc.vector.memset(onesr_sb[:], 1.0)
            nc.vector.memset(warm_sb[:], 0.0)
            # pull the exp table load off the critical path
            nc.scalar.activation(warm_sb[:], warm_sb[:], AF.Exp)
            for sc in range(SCK):
                for k in range(KT_E):
                    nc.sync.dma_start(
                        xT_sb[:, k, sc * 512:(sc + 1) * 512],
                        xT_r[:, k, sc * 512:(sc + 1) * 512],
                    )
            nc.sync.dma_start(wo_sb[:], wo.rearrange("(l p) n -> p l n", p=128))

            qT_sb = qpool.tile([128, 2, S], F16)
            kT_sb = qpool.tile([128, 2, S], F16)
            # per (t, head): 128 cols = [v_h (64) | ones (64)] so one matmul
            # yields av rows 0:64 and the replicated softmax denominator
            # rows 64:128 in a single PSUM bank
            v2_sb = qpool.tile([128, TT, HPG, 128], F16)
            avT_sb = qpool.tile([128, 2, S], F16)
            nc.vector.memset(v2_sb[:, :, :, DH:128], 1.0)

            with (
                tc.tile_pool(name="psc", bufs=3, space="PSUM") as psc,
                tc.tile_pool(name="pavz", bufs=2, space="PSUM") as pavz,
            ):
                def proj_sc(dst, w_sb, b_sb, src, nk, l, sc):
                    ssl = slice(sc * 512, (sc + 1) * 512)
                    ps = psc.tile([128, 512], F32, tag="sc",
                                  name=f"pj{dst.tensor.name}_{l}_{sc}")
                    for k in range(nk):
                        nc.tensor.matmul(
                            ps[:],
                            w_sb[:, k, l * 128:(l + 1) * 128],
                            src[:, k, ssl],
                            start=(k == 0), stop=(k == nk - 1),
                        )
                    nc.vector.tensor_tensor(
                        dst[:, l, ssl], ps[:],
                        b_sb[:, l:l + 1].to_broadcast([128, 512]),
                        OP.add,
                    )

                def vproj_t(t):
                    ps = pavz.tile([128, DSL], F32, tag="avz",
                                   name=f"vps{t}")
                    for k in range(KT_C):
                        nc.tensor.matmul(
                            ps[:],
                            ctxT_sb[:, k, t * 128:(t + 1) * 128],
                            wv_sb[:, k, :],
                            start=(k == 0), stop=False,
                        )
                    nc.tensor.matmul(
                        ps[:], onesr_sb[:, :], bv_sb[:, :],
                        start=False, stop=True,
                    )
                    nc.vector.tensor_copy(
                        v2_sb[:, t, :, 0:DH],
                        ps[:].rearrange("p (g d) -> p g d", d=DH),
                    )

                def outproj_st(sc, st):
                    row = (sc * 4 + st) * 128
                    psos = [psc.tile([128, 512], F32, tag="sc",
                                     name=f"po{sc}_{st}_{n}")
                            for n in range(2)]
                    for l in range(2):
                        for n in range(2):
                            nc.tensor.matmul(
                                psos[n][:],
                                avT_sb[:, l, row:row + 128],
                                wo_sb[:, l, n * 512:(n + 1) * 512],
                                start=(l == 0), stop=(l == 1),
                            )
                    for n in range(2):
                        os_sb = ospool.tile([128, 512], F16, tag="os")
                        nc.vector.tensor_copy(os_sb[:], psos[n][:])
                        nc.sync.dma_start(
                            out[row:row + 128, n * 512:(n + 1) * 512],
                            os_sb[:],
                        )

                # ---- filler queue: PE work paced into the ACT-bound
                # attention loop (~1us slack per t-iteration) ----
                fillers = []

                def pop_filler():
                    if fillers:
                        fillers.pop(0)()

                # upfront: kT l0 (DMA-paced on ctxT), V proj, qT l0 sc0
                for sc in range(SCK):
                    proj_sc(kT_sb, wk_sb, bk_sb, ctxT_sb, KT_C, 0, sc)
                for t in range(TT):
                    vproj_t(t)
                proj_sc(qT_sb, wq_sb, bq_sb, xT_sb, KT_E, 0, 0)

                for sc in range(1, SCK):
                    fillers.append(
                        lambda sc=sc: proj_sc(qT_sb, wq_sb, bq_sb,
                                              xT_sb, KT_E, 0, sc))
                for sc in range(SCK):
                    fillers.append(
                        lambda sc=sc: proj_sc(kT_sb, wk_sb, bk_sb,
                                              ctxT_sb, KT_C, 1, sc))
                for sc in range(SCK):
                    fillers.append(
                        lambda sc=sc: proj_sc(qT_sb, wq_sb, bq_sb,
                                              xT_sb, KT_E, 1, sc))

                # ---- software-pipelined attention ----
                # segment = (p, sc); scores+exp of seg overlap AV of prev
                segs = [(p, sc) for p in range(2) for sc in range(SCK)]
                ex_bank = {}    # (p, sc) -> list of ex tiles
                avz_bank = {}   # (p, sc) -> {h: psum tile}

                def emit_scores_exp(p, sc, t):
                    ssl = slice(sc * 512, (sc + 1) * 512)
                    scp = psc.tile([128, 1024], F32, tag="sc",
                                   name=f"sc{sc}_{t}_{p}")
                    for h in range(2):
                        hb = h * DH
                        nc.tensor.matmul(
                            scp[:, h * 512:(h + 1) * 512],
                            kT_sb[hb:hb + DH, p, t * 128:(t + 1) * 128],
                            qT_sb[hb:hb + DH, p, ssl],
                            start=True, stop=True,
                        )
                    ex = expool.tile([128, 1024], F16, tag="ex",
                                     name=f"ex{sc}_{t}_{p}")
                    nc.scalar.activation(ex[:], scp[:], AF.Exp, scale=0.125)
                    ex_bank[(p, sc)].append(ex)

                def emit_av(p, sc, t):
                    ex = ex_bank[(p, sc)][t]
                    for h in range(2):
                        nc.tensor.matmul(
                            avz_bank[(p, sc)][h][:, :],
                            v2_sb[:, t, p * 2 + h, :],
                            ex[:, h * 512:(h + 1) * 512],
                            start=(t == 0), stop=(t == TT - 1),
                        )

                def normalize(p, sc):
                    ssl = slice(sc * 512, (sc + 1) * 512)
                    for h in range(2):
                        hb = h * DH
                        avz = avz_bank[(p, sc)][h]
                        # custom DVE op: SBUF-only, partition base 0
                        rz = ospool.tile([128, 1024], F32, tag="rz",
                                         name=f"rz{sc}_{p}_{h}")
                        nc.vector.tensor_copy(
                            rz[0:DH, 0:512], avz[DH:128, :])
                        nc.vector.reciprocal_approx_fast(
                            rz[0:DH, 512:1024], rz[0:DH, 0:512])
                        nc.vector.tensor_tensor(
                            avT_sb[hb:hb + DH, p, ssl],
                            avz[0:DH, :],
                            rz[0:DH, 512:1024],
                            OP.mult,
                        )
                    del avz_bank[(p, sc)]
                    del ex_bank[(p, sc)]

                prev = None
                for seg in segs:
                    p, sc = seg
                    ex_bank[seg] = []
                    for t in range(TT):
                        emit_scores_exp(p, sc, t)
                        if prev is not None:
                            if t == 0:
                                avz_bank[prev] = {
                                    h: pavz.tile(
                                        [128, 512], F32, tag="avz",
                                        name=f"avz{prev[1]}_{prev[0]}_{h}")
                                    for h in range(2)}
                            emit_av(prev[0], prev[1], t)
                        pop_filler()
                    if prev is not None:
                        normalize(prev[0], prev[1])
                        if prev[0] == 1:
                            for st in range(4):
                                fillers.append(
                                    lambda sc=prev[1], st=st:
                                    outproj_st(sc, st))
                    prev = seg

                # drain: AV + normalize of the last segment, then any
                # remaining fillers (last out-projs)
                avz_bank[prev] = {
                    h: pavz.tile([128, 512], F32, tag="avz",
                                 name=f"avz{prev[1]}_{prev[0]}_{h}")
                    for h in range(2)}
                for t in range(TT):
                    emit_av(prev[0], prev[1], t)
                    pop_filler()
                normalize(prev[0], prev[1])
                for st in range(4):
                    outproj_st(SCK - 1, st)
                while fillers:
                    pop_filler()

    nc.compile()
    return nc


def get_nc():
    if "nc" not in _NC_CACHE:
        _NC_CACHE["nc"] = _build_nc()
    return _NC_CACHE["nc"]


def make_in_maps(x, context, Wq, bq, Wk, bk, Wv, bv, Wo, bo):
    x = np.asarray(x, dtype=np.float32)
    context = np.asarray(context, dtype=np.float32)
    Wq = np.asarray(Wq, dtype=np.float32)
    Wk = np.asarray(Wk, dtype=np.float32)
    Wv = np.asarray(Wv, dtype=np.float32)
    Wo = np.asarray(Wo, dtype=np.float32)
    bq = np.asarray(bq, dtype=np.float32)
    bk = np.asarray(bk, dtype=np.float32)
    bv = np.asarray(bv, dtype=np.float32)

    xT = [np.ascontiguousarray(x[b].T).astype(np.float16) for b in range(B)]
    ctxT = [np.ascontiguousarray(context[b].T).astype(np.float16)
            for b in range(B)]
    in_maps = []
    for c in range(N_CORES):
        b, g = c // GROUPS, c % GROUPS
        sl = slice(g * DSL, (g + 1) * DSL)
        in_maps.append({
            "xT": xT[b],
            "ctxT": ctxT[b],
            "wq": Wq[:, sl].astype(np.float16),
            "wk": Wk[:, sl].astype(np.float16),
            "wv": Wv[:, sl].astype(np.float16),
            "wo": Wo[sl, :].astype(np.float16),
            "bq": np.ascontiguousarray(bq[sl].reshape(2, 128).T),
            "bk": np.ascontiguousarray(bk[sl].reshape(2, 128).T),
            "bv": bv[sl].reshape(1, DSL).astype(np.float16),
        })
    return in_maps


def run_sharded(inputs, trace=False):
    nc = get_nc()
    in_maps = make_in_maps(**inputs)
    res = bass_utils.run_bass_kernel_spmd(
        nc, in_maps, core_ids=list(range(N_CORES)), trace=trace,
    )
    bo = np.asarray(inputs["bo"], dtype=np.float32)
    full = np.empty((B, S, E), dtype=np.float32)
    for b in range(B):
        acc = res.results[b * GROUPS]["out"].astype(np.float32)
        for g in range(1, GROUPS):
            acc = acc + res.results[b * GROUPS + g]["out"].astype(np.float32)
        full[b] = acc + bo[None, :]
    return full, res.exec_time_ns


def kernel(**inputs) -> np.ndarray:
    return run_sharded(inputs)[0]


# revision 11
# speedup vs baseline: 1.1522x; 1.0204x over previous
"""Trainium2 Bass kernel for nn_CrossAttention (B=2, S=2048, E=1024, H=16, ctx=768).

Sharding: 4-way tensor-parallel over heads x 2-way data-parallel over batch.
Core c handles batch c//4 and heads 4*(c%4) .. 4*(c%4)+3.

Per-core dataflow (fp16 operands, fp32 PSUM accumulate), software-pipelined
so ScalarE (exp, the throughput floor at ~1us per [128,1024] tile) starts
~20us in and never starves:

  segments = head-pair-major: (p=0, sc=0..3) then (p=1, sc=0..3).
  Per segment t-iteration: score pair (concurrent K=64 matmuls on PE row
  groups 0/64) -> exp (ScalarE) banked into a deep SBUF ex pool, while the
  PREVIOUS segment's AV matmuls consume its banked ex tiles, plus
  budget-paced fine-grained "filler" PE closures (remaining projections /
  V-projection / deferred out-proj) sized under the ~600ns per-iteration
  PE slack.  p=0 needs only layer-0 projections, so kT/qT layer-1 run as
  fillers inside the p=0 segments.

  All DRAM operands are host-prearranged into SBUF-partition-contiguous
  layouts (>=3KB DMA lines); xT additionally in s-chunk column blocks so
  qT l0 sc0 (and with it the first score/exp) is unblocked after ~5MB of
  DMA instead of all 8.75MB.

  av/Z: v_h (cols 0:64) and ones (cols 64:128) col-packed into one PSUM
  bank: rows 0:64 = unnormalized out.T, rows 64:128 = softmax denominator.
  Normalized via a base-0 SBUF staged reciprocal_approx_fast + mixed-base
  multiply.  Out-proj partials DMA'd as fp16; host sums partials + bo.
"""
import numpy as np

import concourse.bass as bass
import concourse.mybir as mybir
import concourse.tile as tile
from concourse import bacc, bass_utils

F16 = mybir.dt.float16
F32 = mybir.dt.float32
AF = mybir.ActivationFunctionType
OP = mybir.AluOpType

B, S, E, C, H, DH = 2, 2048, 1024, 768, 16, 64
N_CORES = 8
GROUPS = 4            # head groups (tensor parallel)
HPG = H // GROUPS     # heads per group = 4
DSL = HPG * DH        # feature slice per core = 256
KT_E = E // 128       # 8 k-tiles for x projections
KT_C = C // 128       # 6 k-tiles for context projections
SCK = S // 512        # 4 s-chunks
TT = S // 128         # 16 t-tiles

_NC_CACHE = {}


def _build_nc():
    nc = bacc.Bacc("TRN2", target_bir_lowering=False, debug=False,
                   num_devices=N_CORES)

    # host-prearranged, partition-contiguous layouts
    xT = nc.dram_tensor("xT", [128, SCK * KT_E * 512], F16,
                        kind="ExternalInput").ap()
    ctxT = nc.dram_tensor("ctxT", [128, KT_C * S], F16,
                          kind="ExternalInput").ap()
    wq = nc.dram_tensor("wq", [128, KT_E * DSL], F16,
                        kind="ExternalInput").ap()
    wk = nc.dram_tensor("wk", [128, KT_C * DSL], F16,
                        kind="ExternalInput").ap()
    wv = nc.dram_tensor("wv", [128, KT_C * DSL], F16,
                        kind="ExternalInput").ap()
    wo = nc.dram_tensor("wo", [128, 2 * E], F16, kind="ExternalInput").ap()
    bq = nc.dram_tensor("bq", [128, 2], F32, kind="ExternalInput").ap()
    bk = nc.dram_tensor("bk", [128, 2], F32, kind="ExternalInput").ap()
    bv = nc.dram_tensor("bv", [1, DSL], F16, kind="ExternalInput").ap()
    out = nc.dram_tensor("out", [S, E], F16, kind="ExternalOutput").ap()

    xT_r = xT.rearrange("p (sc k n) -> p sc k n", sc=SCK, k=KT_E)
    ctxT_r = ctxT.rearrange("p (k s) -> p k s", k=KT_C)

    with tile.TileContext(nc) as tc:
        with (
            tc.tile_pool(name="const", bufs=1) as cpool,
            tc.tile_pool(name="qkv", bufs=1) as qpool,
            tc.tile_pool(name="ex", bufs=20) as expool,
            tc.tile_pool(name="os", bufs=3) as ospool,
        ):
            wq_sb = cpool.tile([128, KT_E, DSL], F16)
            wk_sb = cpool.tile([128, KT_C, DSL], F16)
            wv_sb = cpool.tile([128, KT_C, DSL], F16)
            wo_sb = cpool.tile([128, 2, E], F16)
            bq_sb = cpool.tile([128, 2], F32)
            bk_sb = cpool.tile([128, 2], F32)
            bv_sb = cpool.tile([1, DSL], F16)
            onesr_sb = cpool.tile([1, 128], F16)
            warm_sb = cpool.tile([1, 8], F32)
            ctxT_sb = cpool.tile([128, KT_C, S], F16)
            xT_sb = cpool.tile([128, SCK, KT_E, 512], F16)

            # DMA in first-needed-first order (all contiguous lines):
            # kT-proj deps, V deps, qT-sc0 deps, rest of x, wo last.
            nc.sync.dma_start(wk_sb[:], wk.rearrange("p (k m) -> p k m",
                                                     k=KT_C))
            nc.sync.dma_start(bk_sb[:], bk[:])
            for k in range(KT_C):
                nc.sync.dma_start(ctxT_sb[:, k, :], ctxT_r[:, k, :])
            nc.sync.dma_start(wv_sb[:], wv.rearrange("p (k m) -> p k m",
                                                     k=KT_C))
            nc.sync.dma_start(bv_sb[:], bv[:])
            nc.sync.dma_start(wq_sb[:], wq.rearrange("p (k m) -> p k m",
                                                     k=KT_E))
            nc.sync.dma_start(bq_sb[:], bq[:])
            nc.vector.memset(onesr_sb[:], 1.0)
            nc.vector.memset(warm_sb[:], 0.0)
            # pull the exp table load off the critical path
            nc.scalar.activation(warm_sb[:], warm_sb[:], AF.Exp)
            for sc in range(SCK):
                nc.sync.dma_start(xT_sb[:, sc], xT_r[:, sc])
            nc.sync.dma_start(wo_sb[:], wo.rearrange("p (l n) -> p l n",
                                                     l=2))

            qT_sb = qpool.tile([128, 2, S], F16)
            kT_sb = qpool.tile([128, 2, S], F16)
            # per (t, head): 128 cols = [v_h (64) | ones (64)] so one matmul
            # yields av rows 0:64 and the replicated softmax denominator
            # rows 64:128 in a single PSUM bank
            v2_sb = qpool.tile([128, TT, HPG, 128], F16)
            avT_sb = qpool.tile([128, 2, S], F16)
            nc.vector.memset(v2_sb[:], 1.0)

            with (
                tc.tile_pool(name="psc", bufs=3, space="PSUM") as psc,
                tc.tile_pool(name="pavz", bufs=2, space="PSUM") as pavz,
            ):
                # ---- filler queue: (est_ns, closure) self-contained PE
                # work budget-paced into the ACT-bound attention loop.
                # Every closure allocates AND fully consumes its own PSUM
                # tile (alloc -> last reader emitted within the closure) so
                # pool-slot recycling can never wait on a not-yet-emitted
                # instruction (in-order PE => deadlock otherwise). ----
                fillers = []
                fill_spent = [0.0]

                def proj_sc(dst, w_sb, b_sb, src_fn, nk, l, sc):
                    ssl = slice(sc * 512, (sc + 1) * 512)
                    ps = psc.tile([128, 512], F32, tag="sc",
                                  name=f"pj{dst.tensor.name}_{l}_{sc}")
                    for k in range(nk):
                        nc.tensor.matmul(
                            ps[:],
                            w_sb[:, k, l * 128:(l + 1) * 128],
                            src_fn(k, ssl),
                            start=(k == 0), stop=(k == nk - 1),
                        )
                    nc.vector.tensor_tensor(
                        dst[:, l, ssl], ps[:],
                        b_sb[:, l:l + 1].to_broadcast([128, 512]),
                        OP.add,
                    )

                def q_src(k, ssl):
                    sc = ssl.start // 512
                    return xT_sb[:, sc, k, :]

                def k_src(k, ssl):
                    return ctxT_sb[:, k, ssl]

                def vproj_t(t):
                    # psc (not pavz): the long-lived avz accumulators own
                    # pavz; a vps alloc ringing behind them would wait on a
                    # normalize not yet emitted -> deadlock with in-order PE
                    ps = psc.tile([128, DSL], F32, tag="sc",
                                  name=f"vps{t}")
                    for k in range(KT_C):
                        nc.tensor.matmul(
                            ps[:],
                            ctxT_sb[:, k, t * 128:(t + 1) * 128],
                            wv_sb[:, k, :],
                            start=(k == 0), stop=False,
                        )
                    nc.tensor.matmul(
                        ps[:], onesr_sb[:, :], bv_sb[:, :],
                        start=False, stop=True,
                    )
                    nc.vector.tensor_copy(
                        v2_sb[:, t, :, 0:DH],
                        ps[:].rearrange("p (g d) -> p g d", d=DH),
                    )

                def outproj_st(sc, st):
                    row = (sc * 4 + st) * 128
                    psos = [psc.tile([128, 512], F32, tag="sc",
                                     name=f"po{sc}_{st}_{n}")
                            for n in range(2)]
                    for l in range(2):
                        for n in range(2):
                            nc.tensor.matmul(
                                psos[n][:],
                                avT_sb[:, l, row:row + 128],
                                wo_sb[:, l, n * 512:(n + 1) * 512],
                                start=(l == 0), stop=(l == 1),
                            )
                    for n in range(2):
                        os_sb = ospool.tile([128, 512], F16, tag="os")
                        nc.vector.tensor_copy(os_sb[:], psos[n][:])
                        nc.sync.dma_start(
                            out[row:row + 128, n * 512:(n + 1) * 512],
                            os_sb[:],
                        )

                def pop_fillers(budget_ns):
                    fill_spent[0] += budget_ns
                    while fillers and fill_spent[0] >= fillers[0][0]:
                        est, fn, _ = fillers.pop(0)
                        fill_spent[0] -= est
                        fn()

                def drain(label):
                    """Force-emit fillers up to & incl. `label` (producers
                    must precede their consumers in program order)."""
                    while fillers:
                        if not any(lab == label for _, _, lab in fillers):
                            return
                        est, fn, lab = fillers.pop(0)
                        fill_spent[0] = max(0.0, fill_spent[0] - est)
                        fn()
                        if lab == label:
                            return

                # ---- upfront: kT l0 sc0 (DMA-paced on ctxT), V t0..3,
                # qT l0 sc0; everything else becomes fillers ----
                proj_sc(kT_sb, wk_sb, bk_sb, k_src, KT_C, 0, 0)
                for t in range(4):
                    vproj_t(t)
                proj_sc(qT_sb, wq_sb, bq_sb, q_src, KT_E, 0, 0)

                def F(est, fn, label=None):
                    fillers.append((est, fn, label))

                for sc in range(1, SCK):
                    F(1310, lambda sc=sc: proj_sc(kT_sb, wk_sb, bk_sb,
                                                  k_src, KT_C, 0, sc),
                      f"kTl0_{sc}")
                F(1750, lambda: proj_sc(qT_sb, wq_sb, bq_sb, q_src,
                                        KT_E, 0, 1), "qTl0_1")
                for t in range(4, 10):
                    F(870, lambda t=t: vproj_t(t), f"v{t}")
                F(1750, lambda: proj_sc(qT_sb, wq_sb, bq_sb, q_src,
                                        KT_E, 0, 2), "qTl0_2")
                for t in range(10, TT):
                    F(870, lambda t=t: vproj_t(t), f"v{t}")
                F(1750, lambda: proj_sc(qT_sb, wq_sb, bq_sb, q_src,
                                        KT_E, 0, 3), "qTl0_3")
                for sc in range(SCK):
                    F(1310, lambda sc=sc: proj_sc(kT_sb, wk_sb, bk_sb,
                                                  k_src, KT_C, 1, sc),
                      f"kTl1_{sc}")
                for sc in range(SCK):
                    F(1750, lambda sc=sc: proj_sc(qT_sb, wq_sb, bq_sb,
                                                  q_src, KT_E, 1, sc),
                      f"qTl1_{sc}")

                # ---- software-pipelined attention ----
                segs = [(p, sc) for p in range(2) for sc in range(SCK)]
                ex_bank = {}    # (p, sc) -> list of ex tiles
                avz_bank = {}   # (p, sc) -> {h: psum tile}

                def emit_scores_exp(p, sc, t):
                    ssl = slice(sc * 512, (sc + 1) * 512)
                    scp = psc.tile([128, 1024], F32, tag="sc",
                                   name=f"sc{sc}_{t}_{p}")
                    for h in range(2):
                        hb = h * DH
                        nc.tensor.matmul(
                            scp[:, h * 512:(h + 1) * 512],
                            kT_sb[hb:hb + DH, p, t * 128:(t + 1) * 128],
                            qT_sb[hb:hb + DH, p, ssl],
                            start=True, stop=True,
                        )
                    ex = expool.tile([128, 1024], F16, tag="ex",
                                     name=f"ex{sc}_{t}_{p}")
                    nc.scalar.activation(ex[:], scp[:], AF.Exp, scale=0.125)
                    ex_bank[(p, sc)].append(ex)

                def emit_av(p, sc, t):
                    ex = ex_bank[(p, sc)][t]
                    for h in range(2):
                        nc.tensor.matmul(
                            avz_bank[(p, sc)][h][:, :],
                            v2_sb[:, t, p * 2 + h, :],
                            ex[:, h * 512:(h + 1) * 512],
                            start=(t == 0), stop=(t == TT - 1),
                        )

                def alloc_avz(seg):
                    avz_bank[seg] = {
                        h: pavz.tile([128, 512], F32, tag="avz",
                                     name=f"avz{seg[1]}_{seg[0]}_{h}")
                        for h in range(2)}

                def normalize(p, sc):
                    ssl = slice(sc * 512, (sc + 1) * 512)
                    for h in range(2):
                        hb = h * DH
                        avz = avz_bank[(p, sc)][h]
                        # custom DVE op: SBUF-only, partition base 0
                        rz = ospool.tile([128, 1024], F32, tag="rz",
                                         name=f"rz{sc}_{p}_{h}")
                        nc.vector.tensor_copy(
                            rz[0:DH, 0:512], avz[DH:128, :])
                        nc.vector.reciprocal_approx_fast(
                            rz[0:DH, 512:1024], rz[0:DH, 0:512])
                        nc.vector.tensor_tensor(
                            avT_sb[hb:hb + DH, p, ssl],
                            avz[0:DH, :],
                            rz[0:DH, 512:1024],
                            OP.mult,
                        )
                    del avz_bank[(p, sc)]
                    del ex_bank[(p, sc)]

                prev = None
                for seg in segs:
                    p, sc = seg
                    # deadline drains: producers before their consumers
                    if p == 0 and sc > 0:
                        drain(f"qTl0_{sc}")
                    elif p == 1:
                        drain(f"qTl1_{sc}")   # forces kTl1/v fillers too
                    ex_bank[seg] = []
                    for t in range(TT):
                        if seg == (0, 0) and t in (4, 8, 12):
                            drain(f"kTl0_{t // 4}")  # keys for tile t
                        if prev == (0, 0) and t >= 4:
                            drain(f"v{t}")           # v2[t] for its AV
                        if prev is not None:
                            if t == 0:
                                alloc_avz(prev)
                            emit_av(prev[0], prev[1], t)
                        emit_scores_exp(p, sc, t)
                        pop_fillers(600 if prev is None else 190)
                    if prev is not None:
                        normalize(prev[0], prev[1])
                        if prev[0] == 1:
                            for st in range(4):
                                F(872, lambda sc=prev[1], st=st:
                                  outproj_st(sc, st))
                    prev = seg

                # drain: AV + normalize of the last segment, then the
                # remaining out-projections
                alloc_avz(prev)
                for t in range(TT):
                    emit_av(prev[0], prev[1], t)
                    pop_fillers(430)
                normalize(prev[0], prev[1])
                for st in range(4):
                    outproj_st(SCK - 1, st)
                while fillers:
                    fillers.pop(0)[1]()

    nc.compile()
    return nc


def get_nc():
    if "nc" not in _NC_CACHE:
        _NC_CACHE["nc"] = _build_nc()
    return _NC_CACHE["nc"]


def _part_major(a, kt):
    """[kt*128, m] -> [128, kt, m] contiguous fp16."""
    m = a.shape[1]
    return np.ascontiguousarray(
        a.reshape(kt, 128, m).transpose(1, 0, 2)).astype(np.float16)


def make_in_maps(x, context, Wq, bq, Wk, bk, Wv, bv, Wo, bo):
    x = np.asarray(x, dtype=np.float32)
    context = np.asarray(context, dtype=np.float32)
    Wq = np.asarray(Wq, dtype=np.float32)
    Wk = np.asarray(Wk, dtype=np.float32)
    Wv = np.asarray(Wv, dtype=np.float32)
    Wo = np.asarray(Wo, dtype=np.float32)
    bq = np.asarray(bq, dtype=np.float32)
    bk = np.asarray(bk, dtype=np.float32)
    bv = np.asarray(bv, dtype=np.float32)

    # xT: [E, S] -> [128 p, sc, k, 512]; ctxT: [C, S] -> [128 p, k, s]
    xTh, ctxTh = [], []
    for b in range(B):
        xt = x[b].T.reshape(KT_E, 128, SCK, 512).transpose(1, 2, 0, 3)
        xTh.append(np.ascontiguousarray(xt).astype(np.float16).reshape(
            128, -1))
        ct = context[b].T.reshape(KT_C, 128, S).transpose(1, 0, 2)
        ctxTh.append(np.ascontiguousarray(ct).astype(np.float16).reshape(
            128, -1))
    in_maps = []
    for c in range(N_CORES):
        b, g = c // GROUPS, c % GROUPS
        sl = slice(g * DSL, (g + 1) * DSL)
        in_maps.append({
            "xT": xTh[b],
            "ctxT": ctxTh[b],
            "wq": _part_major(Wq[:, sl], KT_E).reshape(128, -1),
            "wk": _part_major(Wk[:, sl], KT_C).reshape(128, -1),
            "wv": _part_major(Wv[:, sl], KT_C).reshape(128, -1),
            "wo": _part_major(Wo[sl, :], 2).reshape(128, -1),
            "bq": np.ascontiguousarray(bq[sl].reshape(2, 128).T),
            "bk": np.ascontiguousarray(bk[sl].reshape(2, 128).T),
            "bv": bv[sl].reshape(1, DSL).astype(np.float16),
        })
    return in_maps


def run_sharded(inputs, trace=False):
    nc = get_nc()
    in_maps = make_in_maps(**inputs)
    res = bass_utils.run_bass_kernel_spmd(
        nc, in_maps, core_ids=list(range(N_CORES)), trace=trace,
    )
    bo = np.asarray(inputs["bo"], dtype=np.float32)
    full = np.empty((B, S, E), dtype=np.float32)
    for b in range(B):
        acc = res.results[b * GROUPS]["out"].astype(np.float32)
        for g in range(1, GROUPS):
            acc = acc + res.results[b * GROUPS + g]["out"].astype(np.float32)
        full[b] = acc + bo[None, :]
    return full, res.exec_time_ns


def kernel(**inputs) -> np.ndarray:
    return run_sharded(inputs)[0]


# revision 14
# speedup vs baseline: 1.1826x; 1.0264x over previous
"""Trainium2 Bass kernel for nn_CrossAttention (B=2, S=2048, E=1024, H=16, ctx=768).

Sharding: 4-way tensor-parallel over heads x 2-way data-parallel over batch.
Core c handles batch c//4 and heads 4*(c%4) .. 4*(c%4)+3.

Per-core dataflow (fp16 operands, fp32 PSUM accumulate), software-pipelined
so ScalarE (exp, the throughput floor at ~1us per [128,1024] tile) starts
~20us in and never starves:

  segments = head-pair-major: (p=0, sc=0..3) then (p=1, sc=0..3).
  Per segment t-iteration: score pair (concurrent K=64 matmuls on PE row
  groups 0/64) -> exp (ScalarE) banked into a deep SBUF ex pool, while the
  PREVIOUS segment's AV matmuls consume its banked ex tiles, plus
  budget-paced fine-grained "filler" PE closures (remaining projections /
  V-projection / deferred out-proj) sized under the ~600ns per-iteration
  PE slack.  p=0 needs only layer-0 projections, so kT/qT layer-1 run as
  fillers inside the p=0 segments.

  All DRAM operands are host-prearranged into SBUF-partition-contiguous
  layouts (>=3KB DMA lines); xT additionally in s-chunk column blocks so
  qT l0 sc0 (and with it the first score/exp) is unblocked after ~5MB of
  DMA instead of all 8.75MB.

  av/Z: v_h (cols 0:64) and ones (cols 64:128) col-packed into one PSUM
  bank: rows 0:64 = unnormalized out.T, rows 64:128 = softmax denominator.
  Normalized via a base-0 SBUF staged reciprocal_approx_fast + mixed-base
  multiply.  Out-proj partials DMA'd as fp16; host sums partials + bo.
"""
import numpy as np

import concourse.bass as bass
import concourse.mybir as mybir
import concourse.tile as tile
from concourse import bacc, bass_utils

F16 = mybir.dt.float16
F32 = mybir.dt.float32
AF = mybir.ActivationFunctionType
OP = mybir.AluOpType

B, S, E, C, H, DH = 2, 2048, 1024, 768, 16, 64
N_CORES = 8
GROUPS = 4            # head groups (tensor parallel)
HPG = H // GROUPS     # heads per group = 4
DSL = HPG * DH        # feature slice per core = 256
KT_E = E // 128       # 8 k-tiles for x projections
KT_C = C // 128       # 6 k-tiles for context projections
SCK = S // 512        # 4 s-chunks
TT = S // 128         # 16 t-tiles

_NC_CACHE = {}


def _build_nc():
    nc = bacc.Bacc("TRN2", target_bir_lowering=False, debug=False,
                   num_devices=N_CORES)

    # host-prearranged, partition-contiguous layouts
    xT = nc.dram_tensor("xT", [128, SCK * KT_E * 512], F16,
                        kind="ExternalInput").ap()
    ctxT = nc.dram_tensor("ctxT", [128, KT_C * S], F16,
                          kind="ExternalInput").ap()
    wq = nc.dram_tensor("wq", [128, KT_E * DSL], F16,
                        kind="ExternalInput").ap()
    wk = nc.dram_tensor("wk", [128, KT_C * DSL], F16,
                        kind="ExternalInput").ap()
    wv = nc.dram_tensor("wv", [128, KT_C * DSL], F16,
                        kind="ExternalInput").ap()
    wo = nc.dram_tensor("wo", [128, 2 * E], F16, kind="ExternalInput").ap()
    bq = nc.dram_tensor("bq", [128, 2], F32, kind="ExternalInput").ap()
    bk = nc.dram_tensor("bk", [128, 2], F32, kind="ExternalInput").ap()
    bv = nc.dram_tensor("bv", [1, DSL], F16, kind="ExternalInput").ap()
    out = nc.dram_tensor("out", [S, E], F16, kind="ExternalOutput").ap()

    xT_r = xT.rearrange("p (sc k n) -> p sc k n", sc=SCK, k=KT_E)
    ctxT_r = ctxT.rearrange("p (k s) -> p k s", k=KT_C)

    with tile.TileContext(nc) as tc:
        with (
            tc.tile_pool(name="const", bufs=1) as cpool,
            tc.tile_pool(name="qkv", bufs=1) as qpool,
            tc.tile_pool(name="ex", bufs=20) as expool,
            tc.tile_pool(name="os", bufs=3) as ospool,
        ):
            wq_sb = cpool.tile([128, KT_E, DSL], F16)
            wk_sb = cpool.tile([128, KT_C, DSL], F16)
            wv_sb = cpool.tile([128, KT_C, DSL], F16)
            wo_sb = cpool.tile([128, 2, E], F16)
            bq_sb = cpool.tile([128, 2], F32)
            bk_sb = cpool.tile([128, 2], F32)
            bv_sb = cpool.tile([1, DSL], F16)
            onesr_sb = cpool.tile([1, 128], F16)
            warm_sb = cpool.tile([1, 8], F32)
            ctxT_sb = cpool.tile([128, KT_C, S], F16)
            xT_sb = cpool.tile([128, SCK, KT_E, 512], F16)

            # DMA in first-needed-first order (all contiguous lines):
            # kT-proj deps, V deps, qT-sc0 deps, rest of x, wo last.
            scratch_sb = cpool.tile([128, 512], F16)

            nc.sync.dma_start(wk_sb[:], wk.rearrange("p (k m) -> p k m",
                                                     k=KT_C))
            nc.sync.dma_start(bk_sb[:], bk[:])
            for k in range(KT_C):
                nc.sync.dma_start(ctxT_sb[:, k, :], ctxT_r[:, k, :])
            nc.sync.dma_start(wq_sb[:], wq.rearrange("p (k m) -> p k m",
                                                     k=KT_E))
            nc.sync.dma_start(bq_sb[:], bq[:])
            nc.sync.dma_start(xT_sb[:, 0], xT_r[:, 0])
            nc.sync.dma_start(wv_sb[:], wv.rearrange("p (k m) -> p k m",
                                                     k=KT_C))
            nc.sync.dma_start(bv_sb[:], bv[:])
            nc.vector.memset(scratch_sb[:], 0.0)
            nc.vector.memset(onesr_sb[:], 1.0)
            nc.vector.memset(warm_sb[:], 0.0)
            # pull the exp table load off the critical path
            nc.scalar.activation(warm_sb[:], warm_sb[:], AF.Exp)
            for sc in range(1, SCK):
                nc.sync.dma_start(xT_sb[:, sc], xT_r[:, sc])
            nc.sync.dma_start(wo_sb[:], wo.rearrange("p (l n) -> p l n",
                                                     l=2))

            qT_sb = qpool.tile([128, 2, S], F16)
            kT_sb = qpool.tile([128, 2, S], F16)
            # per (t, head): 128 cols = [v_h (64) | ones (64)] so one matmul
            # yields av rows 0:64 and the replicated softmax denominator
            # rows 64:128 in a single PSUM bank
            v2_sb = qpool.tile([128, TT, HPG, 128], F16)
            avT_sb = qpool.tile([128, 2, S], F16)
            nc.vector.memset(v2_sb[:], 1.0)

            with (
                tc.tile_pool(name="psc", bufs=3, space="PSUM") as psc,
                tc.tile_pool(name="pavz", bufs=2, space="PSUM") as pavz,
            ):
                # ---- filler queue: (est_ns, closure) self-contained PE
                # work budget-paced into the ACT-bound attention loop.
                # Every closure allocates AND fully consumes its own PSUM
                # tile (alloc -> last reader emitted within the closure) so
                # pool-slot recycling can never wait on a not-yet-emitted
                # instruction (in-order PE => deadlock otherwise). ----
                fillers = []
                fill_spent = [0.0]

                def proj_sc(dst, w_sb, b_sb, src_fn, nk, l, sc):
                    ssl = slice(sc * 512, (sc + 1) * 512)
                    ps = psc.tile([128, 512], F32, tag="sc",
                                  name=f"pj{dst.tensor.name}_{l}_{sc}")
                    for k in range(nk):
                        nc.tensor.matmul(
                            ps[:],
                            w_sb[:, k, l * 128:(l + 1) * 128],
                            src_fn(k, ssl),
                            start=(k == 0), stop=(k == nk - 1),
                        )
                    nc.vector.tensor_tensor(
                        dst[:, l, ssl], ps[:],
                        b_sb[:, l:l + 1].to_broadcast([128, 512]),
                        OP.add,
                    )

                def q_src(k, ssl):
                    sc = ssl.start // 512
                    return xT_sb[:, sc, k, :]

                def k_src(k, ssl):
                    return ctxT_sb[:, k, ssl]

                def vproj_t(t):
                    # psc (not pavz): the long-lived avz accumulators own
                    # pavz; a vps alloc ringing behind them would wait on a
                    # normalize not yet emitted -> deadlock with in-order PE
                    ps = psc.tile([128, DSL], F32, tag="sc",
                                  name=f"vps{t}")
                    for k in range(KT_C):
                        nc.tensor.matmul(
                            ps[:],
                            ctxT_sb[:, k, t * 128:(t + 1) * 128],
                            wv_sb[:, k, :],
                            start=(k == 0), stop=False,
                        )
                    nc.tensor.matmul(
                        ps[:], onesr_sb[:, :], bv_sb[:, :],
                        start=False, stop=True,
                    )
                    nc.vector.tensor_copy(
                        v2_sb[:, t, :, 0:DH],
                        ps[:].rearrange("p (g d) -> p g d", d=DH),
                    )

                def outproj_st(sc, st):
                    row = (sc * 4 + st) * 128
                    psos = [psc.tile([128, 512], F32, tag="sc",
                                     name=f"po{sc}_{st}_{n}")
                            for n in range(2)]
                    for l in range(2):
                        for n in range(2):
                            nc.tensor.matmul(
                                psos[n][:],
                                avT_sb[:, l, row:row + 128],
                                wo_sb[:, l, n * 512:(n + 1) * 512],
                                start=(l == 0), stop=(l == 1),
                            )
                    for n in range(2):
                        os_sb = ospool.tile([128, 512], F16, tag="os")
                        nc.vector.tensor_copy(os_sb[:], psos[n][:])
                        nc.sync.dma_start(
                            out[row:row + 128, n * 512:(n + 1) * 512],
                            os_sb[:],
                        )

                def pop_fillers(budget_ns):
                    fill_spent[0] += budget_ns
                    while fillers and fill_spent[0] >= fillers[0][0]:
                        est, fn, _ = fillers.pop(0)
                        fill_spent[0] -= est
                        fn()

                def drain(label):
                    """Force-emit fillers up to & incl. `label` (producers
                    must precede their consumers in program order)."""
                    while fillers:
                        if not any(lab == label for _, _, lab in fillers):
                            return
                        est, fn, lab = fillers.pop(0)
                        fill_spent[0] = max(0.0, fill_spent[0] - est)
                        fn()
                        if lab == label:
                            return

                # ---- upfront ----
                # 1) HAM warm-up: ~11us of dependency-free garbage matmuls
                # bridge the DMA-dead head so real work runs at 2.4 GHz
                # (PE re-throttles to 1.2 GHz after ~3.4us idle otherwise).
                for w in range(10):
                    wps = psc.tile([128, 512], F32, tag="sc",
                                   name=f"warmup{w}")
                    for r in range(4):
                        nc.tensor.matmul(
                            wps[:], scratch_sb[0:128, 0:128],
                            scratch_sb[:, :],
                            start=(r == 0), stop=(r == 3),
                        )
                # 2) kT l0 for ALL s-chunks, k-outer so each arriving ctxT
                # k-tile feeds 4 back-to-back matmuls (rides the DMA)
                kps = [psc.tile([128, 1024], F32, tag="sc",
                                name=f"kT0ps{j}") for j in range(2)]
                for k in range(KT_C):
                    for j in range(2):
                        for i in range(2):
                            sc = j * 2 + i
                            nc.tensor.matmul(
                                kps[j][:, i * 512:(i + 1) * 512],
                                wk_sb[:, k, 0:128],
                                ctxT_sb[:, k, sc * 512:(sc + 1) * 512],
                                start=(k == 0), stop=(k == KT_C - 1),
                            )
                for j in range(2):
                    nc.vector.tensor_tensor(
                        kT_sb[:, 0, j * 1024:(j + 1) * 1024], kps[j][:],
                        bk_sb[:, 0:1].to_broadcast([128, 1024]),
                        OP.add,
                    )
                # 3) qT l0 sc0 -> unblocks the first score/exp
                proj_sc(qT_sb, wq_sb, bq_sb, q_src, KT_E, 0, 0)

                def F(est, fn, label=None):
                    fillers.append((est, fn, label))

                for t in range(4):
                    F(870, lambda t=t: vproj_t(t), f"v{t}")
                F(1750, lambda: proj_sc(qT_sb, wq_sb, bq_sb, q_src,
                                        KT_E, 0, 1), "qTl0_1")
                for t in range(4, 10):
                    F(870, lambda t=t: vproj_t(t), f"v{t}")
                F(1750, lambda: proj_sc(qT_sb, wq_sb, bq_sb, q_src,
                                        KT_E, 0, 2), "qTl0_2")
                for t in range(10, TT):
                    F(870, lambda t=t: vproj_t(t), f"v{t}")
                F(1750, lambda: proj_sc(qT_sb, wq_sb, bq_sb, q_src,
                                        KT_E, 0, 3), "qTl0_3")
                for sc in range(SCK):
                    F(1310, lambda sc=sc: proj_sc(kT_sb, wk_sb, bk_sb,
                                                  k_src, KT_C, 1, sc),
                      f"kTl1_{sc}")
                for sc in range(SCK):
                    F(1750, lambda sc=sc: proj_sc(qT_sb, wq_sb, bq_sb,
                                                  q_src, KT_E, 1, sc),
                      f"qTl1_{sc}")

                # ---- software-pipelined attention ----
                segs = [(p, sc) for p in range(2) for sc in range(SCK)]
                ex_bank = {}    # (p, sc) -> list of ex tiles
                avz_bank = {}   # (p, sc) -> {h: psum tile}

                def emit_scores_exp(p, sc, t):
                    ssl = slice(sc * 512, (sc + 1) * 512)
                    scp = psc.tile([128, 1024], F32, tag="sc",
                                   name=f"sc{sc}_{t}_{p}")
                    for h in range(2):
                        hb = h * DH
                        nc.tensor.matmul(
                            scp[:, h * 512:(h + 1) * 512],
                            kT_sb[hb:hb + DH, p, t * 128:(t + 1) * 128],
                            qT_sb[hb:hb + DH, p, ssl],
                            start=True, stop=True,
                        )
                    ex = expool.tile([128, 1024], F16, tag="ex",
                                     name=f"ex{sc}_{t}_{p}")
                    nc.scalar.activation(ex[:], scp[:], AF.Exp, scale=0.125)
                    ex_bank[(p, sc)].append(ex)

                def emit_av(p, sc, t):
                    ex = ex_bank[(p, sc)][t]
                    for h in range(2):
                        nc.tensor.matmul(
                            avz_bank[(p, sc)][h][:, :],
                            v2_sb[:, t, p * 2 + h, :],
                            ex[:, h * 512:(h + 1) * 512],
                            start=(t == 0), stop=(t == TT - 1),
                        )

                def alloc_avz(seg):
                    avz_bank[seg] = {
                        h: pavz.tile([128, 512], F32, tag="avz",
                                     name=f"avz{seg[1]}_{seg[0]}_{h}")
                        for h in range(2)}

                def normalize(p, sc):
                    ssl = slice(sc * 512, (sc + 1) * 512)
                    for h in range(2):
                        hb = h * DH
                        avz = avz_bank[(p, sc)][h]
                        # custom DVE op: SBUF-only, partition base 0
                        rz = ospool.tile([128, 1024], F32, tag="rz",
                                         name=f"rz{sc}_{p}_{h}")
                        nc.vector.tensor_copy(
                            rz[0:DH, 0:512], avz[DH:128, :])
                        nc.vector.reciprocal_approx_fast(
                            rz[0:DH, 512:1024], rz[0:DH, 0:512])
                        nc.vector.tensor_tensor(
                            avT_sb[hb:hb + DH, p, ssl],
                            avz[0:DH, :],
                            rz[0:DH, 512:1024],
                            OP.mult,
                        )
                    del avz_bank[(p, sc)]
                    del ex_bank[(p, sc)]

                prev = None
                for seg in segs:
                    p, sc = seg
                    # deadline drains: producers before their consumers
                    if p == 0 and sc > 0:
                        drain(f"qTl0_{sc}")
                    elif p == 1:
                        drain(f"qTl1_{sc}")   # forces kTl1/v fillers too
                    ex_bank[seg] = []
                    for t in range(TT):
                        if prev == (0, 0):
                            drain(f"v{t}")           # v2[t] for its AV
                        if prev is not None:
                            if t == 0:
                                alloc_avz(prev)
                            emit_av(prev[0], prev[1], t)
                        emit_scores_exp(p, sc, t)
                        pop_fillers(600 if prev is None else 190)
                    if prev is not None:
                        normalize(prev[0], prev[1])
                        if prev[0] == 1:
                            for st in range(4):
                                F(872, lambda sc=prev[1], st=st:
                                  outproj_st(sc, st))
                    prev = seg

                # drain: AV + normalize of the last segment, then the
                # remaining out-projections
                alloc_avz(prev)
                for t in range(TT):
                    emit_av(prev[0], prev[1], t)
                    pop_fillers(430)
                normalize(prev[0], prev[1])
                for st in range(4):
                    outproj_st(SCK - 1, st)
                while fillers:
                    fillers.pop(0)[1]()

    nc.compile()
    return nc


def get_nc():
    if "nc" not in _NC_CACHE:
        _NC_CACHE["nc"] = _build_nc()
    return _NC_CACHE["nc"]


def _part_major(a, kt):
    """[kt*128, m] -> [128, kt, m] contiguous fp16."""
    m = a.shape[1]
    return np.ascontiguousarray(
        a.reshape(kt, 128, m).transpose(1, 0, 2)).astype(np.float16)


def make_in_maps(x, context, Wq, bq, Wk, bk, Wv, bv, Wo, bo):
    x = np.asarray(x, dtype=np.float32)
    context = np.asarray(context, dtype=np.float32)
    Wq = np.asarray(Wq, dtype=np.float32)
    Wk = np.asarray(Wk, dtype=np.float32)
    Wv = np.asarray(Wv, dtype=np.float32)
    Wo = np.asarray(Wo, dtype=np.float32)
    bq = np.asarray(bq, dtype=np.float32)
    bk = np.asarray(bk, dtype=np.float32)
    bv = np.asarray(bv, dtype=np.float32)

    # xT: [E, S] -> [128 p, sc, k, 512]; ctxT: [C, S] -> [128 p, k, s]
    xTh, ctxTh = [], []
    for b in range(B):
        xt = x[b].T.reshape(KT_E, 128, SCK, 512).transpose(1, 2, 0, 3)
        xTh.append(np.ascontiguousarray(xt).astype(np.float16).reshape(
            128, -1))
        ct = context[b].T.reshape(KT_C, 128, S).transpose(1, 0, 2)
        ctxTh.append(np.ascontiguousarray(ct).astype(np.float16).reshape(
            128, -1))
    in_maps = []
    for c in range(N_CORES):
        b, g = c // GROUPS, c % GROUPS
        sl = slice(g * DSL, (g + 1) * DSL)
        in_maps.append({
            "xT": xTh[b],
            "ctxT": ctxTh[b],
            "wq": _part_major(Wq[:, sl], KT_E).reshape(128, -1),
            "wk": _part_major(Wk[:, sl], KT_C).reshape(128, -1),
            "wv": _part_major(Wv[:, sl], KT_C).reshape(128, -1),
            "wo": _part_major(Wo[sl, :], 2).reshape(128, -1),
            "bq": np.ascontiguousarray(bq[sl].reshape(2, 128).T),
            "bk": np.ascontiguousarray(bk[sl].reshape(2, 128).T),
            "bv": bv[sl].reshape(1, DSL).astype(np.float16),
        })
    return in_maps


def run_sharded(inputs, trace=False):
    nc = get_nc()
    in_maps = make_in_maps(**inputs)
    res = bass_utils.run_bass_kernel_spmd(
        nc, in_maps, core_ids=list(range(N_CORES)), trace=trace,
    )
    bo = np.asarray(inputs["bo"], dtype=np.float32)
    full = np.empty((B, S, E), dtype=np.float32)
    for b in range(B):
        acc = res.results[b * GROUPS]["out"].astype(np.float32)
        for g in range(1, GROUPS):
            acc = acc + res.results[b * GROUPS + g]["out"].astype(np.float32)
        full[b] = acc + bo[None, :]
    return full, res.exec_time_ns


def kernel(**inputs) -> np.ndarray:
    return run_sharded(inputs)[0]
